# revision 1
# baseline (speedup 1.0000x reference)
"""v4: feature-major LSTM cell kernel, fp16 single-pass + host layout-L.

Sharding: pure data parallel, batch split across 8 cores; tiny weights
replicated.

Host prep per full batch (sharding slices columns):
  xh  [49, B]  fp16 : [x|h|ones].T   (single fp16, no split precision --
                      the 2e-2 gate leaves plenty of room)
  cLL [128,B/4] fp16 : c pre-permuted to layout-L: cLL[32q+h, g*512+t]
                      = c[g*2048 + q*512 + t, h]
  w   [49, 128] fp16 : [Wx; Wh; b]
Outputs hLL,cnLL [128, R/4] fp16 are un-permuted back on host.

Device, per outer group (GRP=8192 rows = 4 blocks x 2048):
  - one 2D DMA each for xh [49,8192] (16KB runs), cLL slice [128,2048]
    (4KB runs); outputs staged in SBUF and written with one 2D DMA each
    per group (4KB runs). No 3D/gpsimd DMA at all.
  - per 2048-row block: 16 fp16 matmuls (4 gates x 4 chunks of 512),
    tile_position col-packed -> layout-L psum IFO [128,3,512] + G
    [128,512]; partition p = 32*chunk + h.
  - tail (all fp16, full 128-lane): ACT sigmoid(IFO), tanh(G);
    DVE m1=I*G, m2=F*C, cn=m1+m2 -> staged; ACT tanh(cn); DVE hn=O*tc.
"""

import sys

if "/opt/trn_rl_repo" not in sys.path:
    sys.path.insert(0, "/opt/trn_rl_repo")

import ml_dtypes
import numpy as np

import bass_rust
import concourse.bass as bass
import concourse.tile as tile
from concourse import mybir

# The gpsimd (Pool) engine takes ~6.4us to boot before its first
# instruction. Stock Bass.__init__ emits the const-AP memsets on gpsimd
# followed by an all-engine barrier, so every engine idles ~6.4us at
# kernel start. Emit those memsets on the vector engine instead and keep
# Pool out of the init barrier (gpsimd work later in the kernel is
# ordered by its own data-dependency semaphores).
_orig_gpsimd_memset = bass.BassGpSimd.memset
_orig_barrier = bass.Bass.all_engine_barrier


def _vector_memset(self, ap, constant):
    return self.bass.vector.memset(ap, constant)


def _barrier_no_pool(self, *, sem_only=False):
    if sem_only:
        return _orig_barrier(self, sem_only=True)
    self.multi_engine_barrier(
        [e for e in self.engines if e != mybir.EngineType.Pool]
    )


def _make_bass():
    # v7 experiment: rerouting the init const memsets off gpsimd and
    # dropping Pool from the init barrier made startup WORSE (first
    # sigmoid 7.4us -> 20.3us) -- the ~7us start latency is generic
    # runtime init, not gpsimd boot. Plain Bass() it is.
    return bass.Bass()

F32 = mybir.dt.float32
F16 = mybir.dt.float16
AF = mybir.ActivationFunctionType

B = 1048576
N_CORES = 8
R = B // N_CORES
IN_DIM, H_DIM = 16, 32
XH = IN_DIM + H_DIM
K_AUG = XH + 1  # 49
G4 = 4 * H_DIM  # 128
P = 128
TF = 512  # rows per chunk (matmul free dim, psum bank width)
NQ = 4  # chunks per block
BLK = NQ * TF  # 2048 rows per block (one psum round)
NB = 4  # blocks per outer group
GRP = NB * BLK  # 8192 rows per outer group (DMA granularity)

# gate -> psum slot in IFOG [., slot, .]; host scales W_g by 2 so that
# tanh(g) = 2*sigmoid(2g) - 1 can be recovered from the fused sigmoid.
GATE_SLOT = {"i": 0, "f": 1, "o": 2, "g": 3}
GATE_COLS = {"i": (0, 32), "f": (32, 64), "g": (64, 96), "o": (96, 128)}


def _split_waits(nc, max_waits=1):
    """Walrus codegen allows at most one semaphore wait per instruction.

    Move excess waits onto preceding same-engine EventSemaphore (pure wait)
    instructions; program order on the engine queue makes this equivalent.
    """
    n = 0
    for f in nc.m.functions:
        for blk in f.blocks:
            insts = blk.instructions
            new = []
            for inst in insts:
                si = inst.sync_info
                waits = list(si.on_wait) if si and si.on_wait else []
                if len(waits) > max_waits:
                    excess, keep = waits[:-max_waits], waits[-max_waits:]
                    for j in range(0, len(excess), max_waits):
                        nop = mybir.InstEventSemaphore(
                            name=f"{inst.name}-tw{j}", ins=[], outs=[]
                        )
                        nop.engine = inst.engine
                        nop.sync_info = bass_rust.SyncInfo(
                            on_wait=excess[j : j + max_waits], on_update=[]
                        )
                        new.append(nop)
                        n += 1
                    si.on_wait = keep
                    inst.sync_info = si
                new.append(inst)
            insts[:] = new
    return n


def build_nc(rows=R, split_waits=True):
    assert rows % GRP == 0
    ngrp = rows // GRP
    ucols = rows // NQ  # layout-L free length per core

    nc = _make_bass()
    xh = nc.dram_tensor("xh", [K_AUG, rows], F16, kind="ExternalInput")
    cLL = nc.dram_tensor("cLL", [P, ucols], F16, kind="ExternalInput")
    w = nc.dram_tensor("w", [K_AUG, G4], F16, kind="ExternalInput")
    hLL = nc.dram_tensor("hLL", [P, ucols], F16, kind="ExternalOutput")
    cnLL = nc.dram_tensor("cnLL", [P, ucols], F16, kind="ExternalOutput")

    with tile.TileContext(nc) as tc:
        with (
            tc.tile_pool(name="const", bufs=1) as constp,
            tc.tile_pool(name="io", bufs=3) as iop,
            tc.tile_pool(name="out", bufs=2) as outp,
            tc.tile_pool(name="work", bufs=3) as workp,
            tc.tile_pool(name="psum", bufs=2, space="PSUM") as psump,
        ):
            w_sb = constp.tile([K_AUG, G4], F16, tag="w")
            nc.sync.dma_start(w_sb[:], w[:])

            # one-GROUP software pipeline: tanh(cn) for group it is fused
            # over all 4 blocks and issued during group it+1, so ACT
            # never stalls on the DVE chain and the tanh init cost is
            # amortized 4x. pending = (o_slices, cn_grp, hn_grp, flushes)
            pending = None

            def retire(pending):
                o_sls, p_cn, p_hn, flush = pending
                tc_grp = workp.tile([P, NB, TF], F16, tag="tc")
                nc.scalar.activation(tc_grp[:], p_cn[:], AF.Tanh)
                for pb, o_sl in enumerate(o_sls):
                    nc.vector.tensor_mul(
                        p_hn[:, pb, :], o_sl, tc_grp[:, pb, :]
                    )
                for dst, src in flush:
                    nc.sync.dma_start(dst, src)

            MULT, ADD = mybir.AluOpType.mult, mybir.AluOpType.add

            for it in range(ngrp):
                roff = it * GRP  # row offset of group
                uoff = it * (GRP // NQ)  # layout-L col offset (2048/group)
                xh_sb = iop.tile([K_AUG, GRP], F16, tag="xh")
                if it == 0:
                    # split per block so the first matmuls only wait for
                    # a quarter of the group's xh
                    for b in range(NB):
                        nc.sync.dma_start(
                            xh_sb[:, b * BLK : (b + 1) * BLK],
                            xh[:, roff + b * BLK : roff + (b + 1) * BLK],
                        )
                else:
                    # each dma_start lands on ONE ring; split by partition
                    # rows to get 4-ring parallelism per group while
                    # keeping full-group 16KB descriptor rows (4KB rows
                    # halve per-ring throughput, one whole-group DMA
                    # starves ring parallelism)
                    for r0, r1 in ((0, 13), (13, 25), (25, 37), (37, 49)):
                        nc.sync.dma_start(
                            xh_sb[r0:r1, :], xh[r0:r1, roff : roff + GRP]
                        )
                c_sb = iop.tile([P, NB, TF], F16, tag="c")
                # gpsimd swdge spreads c over rings the sync hwdge input
                # pool doesn't use; group 0 goes via sync because the
                # gpsimd engine takes ~6us to boot
                if it == 0:
                    nc.sync.dma_start(c_sb[:], cLL[:, uoff : uoff + NB * TF])
                else:
                    nc.gpsimd.dma_start(c_sb[:], cLL[:, uoff : uoff + NB * TF])

                cn_grp = outp.tile([P, NB, TF], F16, tag="cn")
                hn_grp = outp.tile([P, NB, TF], F16, tag="hn")
                o_sls = []

                for b in range(NB):
                    ifog_ps = psump.tile([P, 4, TF], F32, tag="ifog")
                    boff = b * BLK
                    for gate in ("i", "f", "g", "o"):
                        c0, c1 = GATE_COLS[gate]
                        slot = GATE_SLOT[gate]
                        for q in range(NQ):
                            rhs = xh_sb[:, boff + q * TF : boff + (q + 1) * TF]
                            nc.tensor.matmul(
                                ifog_ps[32 * q : 32 * q + 32, slot, :],
                                w_sb[:, c0:c1],
                                rhs,
                                start=True,
                                stop=True,
                                tile_position=(0, 32 * q),
                            )

                    # o-slice of block b is read by retire() during the
                    # NEXT group -> up to 5 tiles live; give this tag a
                    # deeper ring
                    ifog_sb = workp.tile([P, 4, TF], F16, tag="ifog_sb", bufs=6)
                    nc.scalar.activation(ifog_sb[:], ifog_ps[:], AF.Sigmoid)

                    # tg = 2*sigmoid(2g) - 1 = tanh(g); fused affine runs
                    # in the DVE fast path, unlike scalar_tensor_tensor
                    tg = workp.tile([P, TF], F16, tag="tg")
                    nc.vector.tensor_scalar(
                        tg[:], ifog_sb[:, 3, :], 2.0, -1.0, MULT, ADD
                    )
                    m1 = workp.tile([P, TF], F16, tag="m1")
                    nc.vector.tensor_mul(m1[:], ifog_sb[:, 0, :], tg[:])
                    m2 = workp.tile([P, TF], F16, tag="m2")
                    nc.vector.tensor_mul(m2[:], ifog_sb[:, 1, :], c_sb[:, b, :])
                    nc.vector.tensor_add(cn_grp[:, b, :], m1[:], m2[:])
                    o_sls.append(ifog_sb[:, 2, :])

                    if b == 0 and pending is not None:
                        retire(pending)

                pending = (
                    o_sls,
                    cn_grp,
                    hn_grp,
                    [
                        (cnLL[:, uoff : uoff + NB * TF], cn_grp[:]),
                        (hLL[:, uoff : uoff + NB * TF], hn_grp[:]),
                    ],
                )

            retire(pending)

    if split_waits:
        _split_waits(nc)
    return nc


def host_prep(x, h, c, Wx, Wh, b):
    """Build full-batch host arrays (sharding slices columns)."""
    n = x.shape[0]
    A = np.empty((K_AUG, n), dtype=np.float32)
    A[0:IN_DIM] = np.asarray(x, np.float32).T
    A[IN_DIM:XH] = np.asarray(h, np.float32).T
    A[XH] = 1.0
    xh_np = A.astype(np.float16)  # [49, n]

    W = np.concatenate(
        [np.asarray(Wx), np.asarray(Wh), np.asarray(b)[None, :]], axis=0
    ).astype(np.float32)  # [49, 128]
    # scale the g-gate columns by 2: device recovers tanh(g) as
    # 2*sigmoid(2g) - 1 from the fused all-gates sigmoid
    W[:, 64:96] *= 2.0
    W = W.astype(np.float16)

    # layout-L permutation: cLL[32q+h, g*512+t] = c[g*2048+q*512+t, h]
    c4 = np.asarray(c, np.float32).reshape(n // BLK, NQ, TF, H_DIM)
    cLL = (
        np.ascontiguousarray(c4.transpose(1, 3, 0, 2))
        .reshape(P, n // NQ)
        .astype(np.float16)
    )
    return xh_np, cLL, W


def unpermute_LL(aLL, rows):
    """Inverse layout-L: [128, rows/4] -> [rows, 32] f32."""
    a4 = np.asarray(aLL).reshape(NQ, H_DIM, rows // BLK, TF)
    return (
        a4.transpose(2, 0, 3, 1).reshape(rows, H_DIM).astype(np.float32)
    )


_NC_CACHE = {}


def _get_nc(rows=R):
    if rows not in _NC_CACHE:
        _NC_CACHE[rows] = build_nc(rows)
    return _NC_CACHE[rows]


def run(x, h, c, Wx, Wh, b, trace=False, rows=R, n_cores=N_CORES):
    """Shard, execute on the 8 cores, gather. Returns (h_new, c_new, results)."""
    from concourse.bass_utils import run_bass_kernel_spmd

    xh_np, cLL_np, w_np = host_prep(x, h, c, Wx, Wh, b)
    nc = _get_nc(rows)
    in_maps = []
    for i in range(n_cores):
        rsl = slice(i * rows, (i + 1) * rows)
        usl = slice(i * (rows // NQ), (i + 1) * (rows // NQ))
        in_maps.append(
            {
                "xh": np.ascontiguousarray(xh_np[:, rsl]),
                "cLL": np.ascontiguousarray(cLL_np[:, usl]),
                "w": w_np,
            }
        )
    res = run_bass_kernel_spmd(nc, in_maps, list(range(n_cores)), trace=trace)
    n = rows * n_cores
    h_new = np.empty((n, H_DIM), dtype=np.float32)
    c_new = np.empty((n, H_DIM), dtype=np.float32)
    for i, r in enumerate(res.results):
        rsl = slice(i * rows, (i + 1) * rows)
        h_new[rsl] = unpermute_LL(r["hLL"], rows)
        c_new[rsl] = unpermute_LL(r["cnLL"], rows)
    return h_new, c_new, res


def kernel(x, h, c, Wx, Wh, b):
    h_new, c_new, _ = run(x, h, c, Wx, Wh, b)
    return h_new, c_new

